# revision 1
# baseline (speedup 1.0000x reference)
"""Multi-head causal attention with RoPE on 8 trn2 cores.

Sharding: core c -> batch b = c // 4, head group g = c % 4 (heads 4g..4g+4).
Each core computes q/k/v projections for its 4 heads, causal attention, and
a partial output-projection (its heads' slice of Wo). The host sums the 4
partials per batch (tensor-parallel unshard) and adds the output bias.

Device layout notes:
  - x is passed transposed with a ones row appended: xT [1025, 2048] bf16.
  - Wq/Wk columns are permuted so the rotary "x1" halves of all 4 heads form
    output partitions 0..127 and the "x2" halves partitions 0..127 of a
    second chunk; RoPE is then 6 full-width vector ops per projection.
  - q/k are produced directly transposed ([d, s]); scores are computed
    transposed ([kk, q]) so the PV matmul consumes them as weights without
    any transpose, and the softmax denominator comes from a ones column
    appended to v (generated by the bias row of Wv).
  - The attention output is normalized with per-partition scalars, moved to
    [dh, s] layout via DMA xbar transposes, and hits the Wo matmul which
    writes the final output transposed ([fo, s]); the host transposes back.
"""

import os

import numpy as np
import ml_dtypes

BF16 = ml_dtypes.bfloat16

B, S, F = 2, 2048, 1024
H, D = 16, 64
HALF = D // 2
NCORES = 8
HPC = 4  # heads per core
S_TILES = S // 128  # 16
N_CH = S // 512  # 4  (512-wide column chunks of s)
F_CH = F // 128  # 8
MACROS = 4  # q macro tiles of 512
MAX_WAVELENGTH = 10000.0

_CACHE = {}
LAST_RESULT = None


def _build_nc():
    import concourse.bacc as bacc
    import concourse.tile as tile
    import concourse.mybir as mybir
    import concourse.bass as bass

    fp32 = mybir.dt.float32
    bf16 = mybir.dt.bfloat16
    MULT = mybir.AluOpType.mult
    ADD = mybir.AluOpType.add
    EXP = mybir.ActivationFunctionType.Exp
    IDENT = mybir.ActivationFunctionType.Identity

    nc = bacc.Bacc("TRN2", target_bir_lowering=False, debug=False)

    xT_d = nc.dram_tensor("xT", [F + 1, S], bf16, kind="ExternalInput")
    wq_d = nc.dram_tensor("wq", [F, 256], bf16, kind="ExternalInput")
    wk_d = nc.dram_tensor("wk", [F, 256], bf16, kind="ExternalInput")
    wv_d = nc.dram_tensor("wv", [F + 1, 260], bf16, kind="ExternalInput")
    wo_d = nc.dram_tensor("wo", [256, F], bf16, kind="ExternalInput")
    bqk_d = nc.dram_tensor("bqk", [128, 4], fp32, kind="ExternalInput")
    cos_d = nc.dram_tensor("cosw", [128, S], bf16, kind="ExternalInput")
    sin_d = nc.dram_tensor("sinw", [128, S], bf16, kind="ExternalInput")
    mask_d = nc.dram_tensor("mask", [128, 256], bf16, kind="ExternalInput")
    outT_d = nc.dram_tensor("outT", [F, S], fp32, kind="ExternalOutput")

    with tile.TileContext(nc) as tc:
        with (
            tc.tile_pool(name="persist", bufs=1) as persist,
            tc.tile_pool(name="tmp", bufs=8) as tmp,
            tc.tile_pool(name="attn", bufs=6) as attn_pool,
            tc.tile_pool(name="ostage", bufs=8) as ostage,
            tc.tile_pool(name="psA", bufs=2, space="PSUM") as psA,
            tc.tile_pool(name="psPV", bufs=4, space="PSUM") as psPV,
        ):
            # ---------------- persistent SBUF tensors + loads ----------
            xT = [persist.tile([128, S], bf16, tag=f"xT{i}", name=f"xT{i}") for i in range(F_CH)]
            xones = persist.tile([1, S], bf16, tag="xones", name="xones")
            for i in range(F_CH):
                nc.sync.dma_start(out=xT[i], in_=xT_d[128 * i : 128 * (i + 1), :])
            nc.sync.dma_start(out=xones, in_=xT_d[F : F + 1, :])

            wq = [persist.tile([128, 256], bf16, tag=f"wq{i}", name=f"wq{i}") for i in range(F_CH)]
            wk = [persist.tile([128, 256], bf16, tag=f"wk{i}", name=f"wk{i}") for i in range(F_CH)]
            wv = [persist.tile([128, 260], bf16, tag=f"wv{i}", name=f"wv{i}") for i in range(F_CH)]
            wvb = persist.tile([1, 260], bf16, tag="wvb", name="wvb")
            for i in range(F_CH):
                nc.sync.dma_start(out=wq[i], in_=wq_d[128 * i : 128 * (i + 1), :])
                nc.sync.dma_start(out=wk[i], in_=wk_d[128 * i : 128 * (i + 1), :])
                nc.sync.dma_start(out=wv[i], in_=wv_d[128 * i : 128 * (i + 1), :])
            nc.sync.dma_start(out=wvb, in_=wv_d[F : F + 1, :])

            wo = [persist.tile([128, F], bf16, tag=f"wo{i}", name=f"wo{i}") for i in range(2)]
            for i in range(2):
                nc.sync.dma_start(out=wo[i], in_=wo_d[128 * i : 128 * (i + 1), :])

            bqk = persist.tile([128, 4], fp32, tag="bqk", name="bqk")
            cosw = persist.tile([128, S], bf16, tag="cosw", name="cosw")
            sinw = persist.tile([128, S], bf16, tag="sinw", name="sinw")
            maskt = persist.tile([128, 256], bf16, tag="maskt", name="maskt")
            nc.sync.dma_start(out=bqk, in_=bqk_d[:, :])
            nc.sync.dma_start(out=cosw, in_=cos_d[:, :])
            nc.sync.dma_start(out=sinw, in_=sin_d[:, :])
            nc.sync.dma_start(out=maskt, in_=mask_d[:, :])

            # post-RoPE q/k, transposed layout [d, s]; chunk 0 = x1 halves
            # of the 4 heads (head h -> partitions 32h..32h+32), chunk 1 = x2.
            q1 = persist.tile([128, S], bf16, tag="q1", name="q1")
            q2 = persist.tile([128, S], bf16, tag="q2", name="q2")
            k1 = persist.tile([128, S], bf16, tag="k1", name="k1")
            k2 = persist.tile([128, S], bf16, tag="k2", name="k2")
            # v in [s, d] layout; head h cols 65h..65h+64, col 65h+64 = ones
            v_sb = [persist.tile([128, 260], bf16, tag=f"v{i}", name=f"v{i}") for i in range(S_TILES)]
            # attention output, [dh, s] layout (head h -> tile h//2 rows 64*(h%2))
            aoT = [persist.tile([128, S], bf16, tag=f"aoT{i}", name=f"aoT{i}") for i in range(2)]

            # ---------------- v projection ------------------------------
            for st in range(S_TILES):
                ps = psA.tile([128, 260], fp32, tag="sc", name="psv")
                sl = slice(128 * st, 128 * (st + 1))
                for kc in range(F_CH):
                    nc.tensor.matmul(ps, xT[kc][:, sl], wv[kc], start=(kc == 0), stop=False)
                nc.tensor.matmul(ps, xones[:, sl], wvb, start=False, stop=True)
                nc.vector.tensor_copy(v_sb[st], ps)

            # ---------------- q/k projections + RoPE --------------------
            for (w_sb, b0, o1, o2) in ((wq, 0, q1, q2), (wk, 2, k1, k2)):
                for n in range(N_CH):
                    nsl = slice(512 * n, 512 * (n + 1))
                    ps1 = psA.tile([128, 512], fp32, tag="sc", name="ps1")
                    ps2 = psA.tile([128, 512], fp32, tag="sc", name="ps2")
                    for kc in range(F_CH):
                        nc.tensor.matmul(ps1, w_sb[kc][:, 0:128], xT[kc][:, nsl],
                                         start=(kc == 0), stop=(kc == F_CH - 1))
                    for kc in range(F_CH):
                        nc.tensor.matmul(ps2, w_sb[kc][:, 128:256], xT[kc][:, nsl],
                                         start=(kc == 0), stop=(kc == F_CH - 1))
                    # drain psum via ACT copies with the bias fused in, then
                    # RoPE as bf16 tensor-tensor ops (2x DVE mode) in SBUF
                    c1 = tmp.tile([128, 512], bf16, tag="rope", name="c1")
                    c2 = tmp.tile([128, 512], bf16, tag="rope", name="c2")
                    nc.scalar.activation(c1, ps1, func=IDENT, bias=bqk[:, b0:b0 + 1])
                    nc.scalar.activation(c2, ps2, func=IDENT, bias=bqk[:, b0 + 1:b0 + 2])
                    t1 = tmp.tile([128, 512], bf16, tag="rope", name="t1")
                    t2 = tmp.tile([128, 512], bf16, tag="rope", name="t2")
                    t3 = tmp.tile([128, 512], bf16, tag="rope", name="t3")
                    t4 = tmp.tile([128, 512], bf16, tag="rope", name="t4")
                    # x1' = x1*cos - x2*sin ; x2' = x2*cos + x1*sin
                    nc.vector.tensor_mul(t1, c1, cosw[:, nsl])
                    nc.vector.tensor_mul(t2, c2, sinw[:, nsl])
                    nc.vector.tensor_mul(t3, c2, cosw[:, nsl])
                    nc.vector.tensor_mul(t4, c1, sinw[:, nsl])
                    nc.vector.tensor_sub(o1[:, nsl], t1, t2)
                    nc.vector.tensor_add(o2[:, nsl], t3, t4)

            # ---------------- attention ---------------------------------
            # scores transposed [kk, q]; two heads share one [128, 1024]
            # psum tile (head pair p: cols 512*hh); exp is one strided ACT op
            # over both heads' causal-valid columns. PV uses v as stationary:
            # out pvT[h] = [65 dh, 512 q] accumulated over kk, row 64 = sums.
            for m in range(MACROS):
                msl = slice(512 * m, 512 * (m + 1))
                pvT = [psPV.tile([65, 512], fp32, tag="pvT", name="pvT") for _ in range(HPC)]
                for kk in range(4 * m + 4):
                    t = kk - 4 * m  # >= 0 -> this kk-chunk holds the diagonal
                    lo = max(0, t) * 128
                    ksl = slice(128 * kk, 128 * (kk + 1))
                    pair_exp = os.environ.get("KVAR_PAIR_EXP", "1") == "1"
                    for p in range(2):
                        if pair_exp:
                            sps = psA.tile([128, 1024], fp32, tag="sc", name="sps")
                            for hh in range(2):
                                h = 2 * p + hh
                                hp = slice(32 * h, 32 * (h + 1))
                                tp = (32 * h, 0)
                                ssl = slice(512 * hh, 512 * hh + 512)
                                qsl = slice(512 * m + lo, 512 * (m + 1))
                                osl = slice(512 * hh + lo, 512 * hh + 512)
                                nc.tensor.matmul(sps[:, osl], k1[hp, ksl], q1[hp, qsl],
                                                 start=True, stop=False, tile_position=tp)
                                nc.tensor.matmul(sps[:, osl], k2[hp, ksl], q2[hp, qsl],
                                                 start=False, stop=True, tile_position=tp)
                            at = attn_pool.tile([128, 1024], bf16, tag="attn", name="at")
                            sps_v = sps[:, :].rearrange("a (h q) -> a h q", h=2)[:, :, lo:512]
                            at_v = at[:, :].rearrange("a (h q) -> a h q", h=2)[:, :, lo:512]
                            nc.scalar.activation(out=at_v, in_=sps_v, func=EXP, scale=0.125)
                            if t >= 0:
                                dv = at[:, :].rearrange("a (h q) -> a h q", h=2)[:, :, 128 * t:128 * (t + 1)]
                                nc.vector.tensor_tensor(dv, dv, maskt, op=MULT)
                            ats_p = [at[:, 0:512], at[:, 512:1024]]
                        else:
                            ats_p = []
                            for hh in range(2):
                                h = 2 * p + hh
                                hp = slice(32 * h, 32 * (h + 1))
                                tp = (32 * h, 0)
                                sps = psA.tile([128, 512], fp32, tag="sc", name="sps")
                                nc.tensor.matmul(sps, k1[hp, ksl], q1[hp, msl],
                                                 start=True, stop=False, tile_position=tp)
                                nc.tensor.matmul(sps, k2[hp, ksl], q2[hp, msl],
                                                 start=False, stop=True, tile_position=tp)
                                at = attn_pool.tile([128, 512], bf16, tag="attn", name="at")
                                nc.scalar.activation(out=at[:, lo:512], in_=sps[:, lo:512],
                                                     func=EXP, scale=0.125)
                                if t >= 0:
                                    dsl = slice(128 * t, 128 * (t + 1))
                                    nc.vector.tensor_tensor(at[:, dsl], at[:, dsl],
                                                            maskt[:, 0:128], op=MULT)
                                ats_p.append(at[:, 0:512])
                        for hh in range(2):
                            h = 2 * p + hh
                            nc.tensor.matmul(
                                pvT[h][:, lo:512],
                                v_sb[kk][:, 65 * h:65 * h + 65],
                                ats_p[hh][:, lo:512],
                                start=(kk == 0), stop=(kk == 4 * m + 3))
                # normalize: rows 0..63 scaled by 1/row64. HW constraints:
                # engine ops need partition base 0 on every operand, so the
                # sums row is extracted with a base-0 full copy, hopped to
                # partition 0 by DMA, reciprocal'd and broadcast at base 0,
                # and the final [dh, s] placement goes through DMA.
                for h in range(HPC):
                    cix, r0 = h // 2, 64 * (h % 2)
                    s65 = tmp.tile([65, 512], fp32, tag="s65", name="s65")
                    nc.vector.tensor_copy(s65, pvT[h][0:65, :])
                    rec0 = tmp.tile([1, 512], fp32, tag="rec0", name="rec0")
                    nc.sync.dma_start(out=rec0, in_=s65[64:65, :])
                    rcp = tmp.tile([1, 512], fp32, tag="rcp", name="rcp")
                    nc.vector.reciprocal_approx_fast(rcp, rec0)
                    rb = tmp.tile([64, 512], fp32, tag="rb", name="rb")
                    nc.gpsimd.partition_broadcast(rb, rcp[0:1, :])
                    ao = ostage.tile([64, 512], bf16, tag="ao", name="ao")
                    # read from s65 (not psum) so the pvT bank frees right
                    # after the copy and the next macro's PV can start
                    nc.vector.tensor_tensor(ao, s65[0:64, :], rb, op=MULT)
                    nc.sync.dma_start(out=aoT[cix][r0:r0 + 64, msl], in_=ao)

            # ---------------- output projection (transposed) ------------
            for fo in range(F_CH):
                fsl = slice(128 * fo, 128 * (fo + 1))
                for sc in range(N_CH):
                    pw = psA.tile([128, 512], fp32, tag="sc", name="pw")
                    for c in range(2):
                        nc.tensor.matmul(pw, wo[c][:, fsl],
                                         aoT[c][:, 512 * sc:512 * (sc + 1)],
                                         start=(c == 0), stop=(c == 1))
                    ow = ostage.tile([128, 512], fp32, tag="ow", name="ow")
                    if sc % 2 == 0:
                        nc.vector.tensor_copy(ow, pw)
                    else:
                        nc.scalar.copy(ow, pw)
                    nc.sync.dma_start(out=outT_d[fsl, 512 * sc:512 * (sc + 1)], in_=ow)

    nc.compile()
    return nc


def _get_nc():
    if "nc" not in _CACHE:
        _CACHE["nc"] = _build_nc()
    return _CACHE["nc"]


def _host_prep(x, positions, Wq, bq, Wk, bk, Wv, bv, Wo, bo):
    """Build the 8 per-core input maps."""
    ts = MAX_WAVELENGTH ** (2.0 * np.arange(HALF, dtype=np.float32) / D)  # [32]
    in_maps = []
    for c in range(NCORES):
        b, g = c // 4, c % 4
        heads = np.arange(4 * g, 4 * g + 4)
        cols_x1 = np.concatenate([64 * h + np.arange(32) for h in heads])
        cols_x2 = cols_x1 + 32
        perm = np.concatenate([cols_x1, cols_x2])

        xT = np.empty((F + 1, S), dtype=BF16)
        xT[:F] = x[b].T.astype(BF16)
        xT[F] = 1.0

        wv_e = np.zeros((F + 1, 260), dtype=np.float32)
        for hl, h in enumerate(heads):
            wv_e[:F, 65 * hl:65 * hl + 64] = Wv[:, 64 * h:64 * h + 64]
            wv_e[F, 65 * hl:65 * hl + 64] = bv[64 * h:64 * h + 64]
            wv_e[F, 65 * hl + 64] = 1.0

        bqk = np.stack([bq[cols_x1], bq[cols_x2], bk[cols_x1], bk[cols_x2]],
                       axis=1).astype(np.float32)  # [128, 4]

        pos = positions[b].astype(np.float32)  # [S]
        ang = pos[None, :] / ts[:, None]  # [32, S]
        cosw = np.tile(np.cos(ang), (4, 1)).astype(BF16)
        sinw = np.tile(np.sin(ang), (4, 1)).astype(BF16)

        ii = np.arange(128)
        mask = np.tile((ii[:, None] <= ii[None, :]).astype(BF16), (1, 2))

        in_maps.append({
            "xT": xT,
            "wq": Wq[:, perm].astype(BF16),
            "wk": Wk[:, perm].astype(BF16),
            "wv": wv_e.astype(BF16),
            "wo": Wo[64 * heads[0]:64 * heads[0] + 256, :].astype(BF16),
            "bqk": bqk,
            "cosw": cosw,
            "sinw": sinw,
            "mask": np.ascontiguousarray(mask),
        })
    return in_maps


def kernel(x, positions, Wq, bq, Wk, bk, Wv, bv, Wo, bo):
    global LAST_RESULT
    from concourse.bass_utils import run_bass_kernel_spmd

    x = np.asarray(x, dtype=np.float32)
    positions = np.asarray(positions)
    args = [np.asarray(a, dtype=np.float32) for a in (Wq, bq, Wk, bk, Wv, bv, Wo, bo)]
    Wq, bq, Wk, bk, Wv, bv, Wo, bo = args

    nc = _get_nc()
    in_maps = _host_prep(x, positions, Wq, bq, Wk, bk, Wv, bv, Wo, bo)
    try:
        res = run_bass_kernel_spmd(nc, in_maps, core_ids=list(range(NCORES)))
    except ModuleNotFoundError:
        # axon NTFF profiling hook unavailable in this image; run untraced
        os.environ["BASS_NEVER_TRACE"] = "1"
        res = run_bass_kernel_spmd(nc, in_maps, core_ids=list(range(NCORES)))
    LAST_RESULT = res

    out = np.empty((B, S, F), dtype=np.float32)
    for b in range(B):
        acc = np.zeros((F, S), dtype=np.float32)
        for g in range(4):
            acc += res.results[4 * b + g]["outT"]
        out[b] = acc.T + bo[None, :]
    return out



# revision 2
# speedup vs baseline: 1.0086x; 1.0086x over previous
"""Multi-head causal attention with RoPE on 8 trn2 cores.

Sharding: core c -> batch b = c // 4, head group g = c % 4 (heads 4g..4g+4).
Each core computes q/k/v projections for its 4 heads, causal attention, and
a partial output-projection (its heads' slice of Wo). The host sums the 4
partials per batch (tensor-parallel unshard) and adds the output bias.

Device layout notes (v2 — PE-utilization-focused rewrite):
  - q/k use an interleaved head layout: tile A = local heads 0,1 and tile B
    = heads 2,3, each head occupying 64 consecutive partitions in natural
    dim order [x1(32); x2(32)].  QK^T is then ONE matmul per head per
    kk-chunk with contraction 64 (vs two K=32 passes before).
  - RoPE: the projection psum is drained to c (bias fused), a permutation
    matmul produces cswap (x1/x2 halves swapped per head), and
    q = c*COS + cswap*SSGN with SSGN = [-sin; +sin] per head: 3 DVE ops.
  - v keeps [s, d] layout; per head the softmax-denominator ones column
    comes FIRST (col 65h), so pv psum row 0 = denominator at partition 0:
    normalization needs no DMA partition hop (reciprocal straight from
    psum row 0, gpsimd broadcast, one multiply).
  - Scores psums are per-head [128, 512] single-bank tiles (4 score bufs +
    4 pv bufs = 8 psum banks) so chunk kk+1's QK can start while chunk
    kk's exps drain: keeps the PE continuously busy (p-state ramp).
  - Phases are interleaved (vproj/kproj/qproj per s-chunk, attention per
    macro, output-projection for macro m emitted after macro m+1) and x is
    DMA'd in s-chunks so the first matmul starts ~1us in.
  - Output is stored as fp16 (halves the output DMA); host sums partials.
"""

import os

import numpy as np
import ml_dtypes

BF16 = ml_dtypes.bfloat16

B, S, F = 2, 2048, 1024
H, D = 16, 64
HALF = D // 2
NCORES = 8
HPC = 4  # heads per core
S_TILES = S // 128  # 16
N_CH = S // 512  # 4  (512-wide column chunks of s)
F_CH = F // 128  # 8
MACROS = 4  # q macro tiles of 512
MAX_WAVELENGTH = 10000.0

_CACHE = {}
LAST_RESULT = None


def _build_nc():
    import concourse.bacc as bacc
    import concourse.tile as tile
    import concourse.mybir as mybir
    import concourse.bass as bass

    fp32 = mybir.dt.float32
    fp16 = mybir.dt.float16
    bf16 = mybir.dt.bfloat16
    MULT = mybir.AluOpType.mult
    EXP = mybir.ActivationFunctionType.Exp
    IDENT = mybir.ActivationFunctionType.Identity

    nc = bacc.Bacc("TRN2", target_bir_lowering=False, debug=False)

    xT_d = nc.dram_tensor("xT", [F + 1, S], bf16, kind="ExternalInput")
    wq_d = nc.dram_tensor("wq", [F, 256], bf16, kind="ExternalInput")
    wk_d = nc.dram_tensor("wk", [F, 256], bf16, kind="ExternalInput")
    wv_d = nc.dram_tensor("wv", [F + 1, 260], bf16, kind="ExternalInput")
    wo_d = nc.dram_tensor("wo", [256, F], bf16, kind="ExternalInput")
    bqk_d = nc.dram_tensor("bqk", [128, 4], fp32, kind="ExternalInput")
    cos_d = nc.dram_tensor("cosw", [128, S], bf16, kind="ExternalInput")
    ssgn_d = nc.dram_tensor("ssgnw", [128, S], bf16, kind="ExternalInput")
    perm_d = nc.dram_tensor("perm", [128, 128], bf16, kind="ExternalInput")
    mask_d = nc.dram_tensor("mask", [128, 128], bf16, kind="ExternalInput")
    outT_d = nc.dram_tensor("outT", [F, S], fp16, kind="ExternalOutput")

    with tile.TileContext(nc) as tc:
        with (
            tc.tile_pool(name="persist", bufs=1) as persist,
            tc.tile_pool(name="tmp", bufs=8) as tmp,
            tc.tile_pool(name="attn", bufs=8) as attn_pool,
            tc.tile_pool(name="ostage", bufs=6) as ostage,
            tc.tile_pool(name="ps", bufs=4, space="PSUM") as psA,
            tc.tile_pool(name="psPV", bufs=4, space="PSUM") as psPV,
        ):
            # ---------------- persistent SBUF tensors -------------------
            xT = [persist.tile([128, S], bf16, tag=f"xT{i}", name=f"xT{i}") for i in range(F_CH)]
            xones = persist.tile([1, S], bf16, tag="xones", name="xones")
            wq = [persist.tile([128, 256], bf16, tag=f"wq{i}", name=f"wq{i}") for i in range(F_CH)]
            wk = [persist.tile([128, 256], bf16, tag=f"wk{i}", name=f"wk{i}") for i in range(F_CH)]
            wv = [persist.tile([128, 260], bf16, tag=f"wv{i}", name=f"wv{i}") for i in range(F_CH)]
            wvb = persist.tile([1, 260], bf16, tag="wvb", name="wvb")
            wo = [persist.tile([128, F], bf16, tag=f"wo{i}", name=f"wo{i}") for i in range(2)]
            bqk = persist.tile([128, 4], fp32, tag="bqk", name="bqk")
            cosw = persist.tile([128, S], bf16, tag="cosw", name="cosw")
            ssgnw = persist.tile([128, S], bf16, tag="ssgnw", name="ssgnw")
            permt = persist.tile([128, 128], bf16, tag="permt", name="permt")
            maskt = persist.tile([128, 128], bf16, tag="maskt", name="maskt")

            # post-RoPE q/k, interleaved layout: tile A heads 0,1 / tile B
            # heads 2,3; head (h%2) at partitions 64*(h%2)..+64, dims natural
            qk_sb = {}
            for nm in ("qA", "qB", "kA", "kB"):
                qk_sb[nm] = persist.tile([128, S], bf16, tag=nm, name=nm)
            # v in [s, d] layout; head h: col 65h = ones, cols 65h+1..+65 = v
            v_sb = [persist.tile([128, 260], bf16, tag=f"v{i}", name=f"v{i}") for i in range(S_TILES)]
            # attention output, [dh, s] layout (head h -> tile h//2 rows 64*(h%2))
            aoT = [persist.tile([128, S], bf16, tag=f"aoT{i}", name=f"aoT{i}") for i in range(2)]

            # ---------------- input DMA, ordered by first use -----------
            for i in range(F_CH):
                nc.sync.dma_start(out=wv[i], in_=wv_d[128 * i : 128 * (i + 1), :])
            nc.sync.dma_start(out=wvb, in_=wv_d[F : F + 1, :])
            nc.sync.dma_start(out=xones, in_=xT_d[F : F + 1, :])
            nc.sync.dma_start(out=bqk, in_=bqk_d[:, :])
            nc.sync.dma_start(out=permt, in_=perm_d[:, :])
            nc.sync.dma_start(out=maskt, in_=mask_d[:, :])
            # x s-chunk 0, then q/k weights + rope tables, then the rest
            for i in range(F_CH):
                nc.sync.dma_start(out=xT[i][:, 0:512], in_=xT_d[128 * i : 128 * (i + 1), 0:512])
            for i in range(F_CH):
                nc.sync.dma_start(out=wk[i], in_=wk_d[128 * i : 128 * (i + 1), :])
                nc.sync.dma_start(out=wq[i], in_=wq_d[128 * i : 128 * (i + 1), :])
            nc.sync.dma_start(out=cosw, in_=cos_d[:, :])
            nc.sync.dma_start(out=ssgnw, in_=ssgn_d[:, :])
            for sc in range(1, N_CH):
                ssl = slice(512 * sc, 512 * (sc + 1))
                for i in range(F_CH):
                    nc.sync.dma_start(out=xT[i][:, ssl], in_=xT_d[128 * i : 128 * (i + 1), ssl])
            for i in range(2):
                nc.sync.dma_start(out=wo[i], in_=wo_d[128 * i : 128 * (i + 1), :])

            # ---------------- phase emitters ----------------------------
            def vproj(st):
                ps = psA.tile([128, 260], fp32, tag="ps", name="psv")
                sl = slice(128 * st, 128 * (st + 1))
                for kc in range(F_CH):
                    nc.tensor.matmul(ps, xT[kc][:, sl], wv[kc], start=(kc == 0), stop=False)
                nc.tensor.matmul(ps, xones[:, sl], wvb, start=False, stop=True)
                nc.vector.tensor_copy(v_sb[st], ps)

            def qkproj(which, n):
                w_sb = wq if which == "q" else wk
                bcol = 0 if which == "q" else 2
                nsl = slice(512 * n, 512 * (n + 1))
                for ti, tn in enumerate("AB"):
                    out = qk_sb[which + tn]
                    csl = slice(128 * ti, 128 * (ti + 1))
                    ps = psA.tile([128, 512], fp32, tag="ps", name="psp")
                    for kc in range(F_CH):
                        nc.tensor.matmul(ps, w_sb[kc][:, csl], xT[kc][:, nsl],
                                         start=(kc == 0), stop=(kc == F_CH - 1))
                    c = tmp.tile([128, 512], bf16, tag="rope", name="c")
                    nc.scalar.activation(c, ps, func=IDENT,
                                         bias=bqk[:, bcol + ti : bcol + ti + 1])
                    pss = psA.tile([128, 512], fp32, tag="ps", name="pss")
                    nc.tensor.matmul(pss, permt, c, start=True, stop=True)
                    t1 = tmp.tile([128, 512], bf16, tag="rope", name="t1")
                    t2 = tmp.tile([128, 512], bf16, tag="rope", name="t2")
                    nc.vector.tensor_mul(t1, c, cosw[:, nsl])
                    nc.vector.tensor_tensor(t2, pss, ssgnw[:, nsl], op=MULT)
                    nc.vector.tensor_add(out[:, nsl], t1, t2)

            def attn_macro(m):
                msl = slice(512 * m, 512 * (m + 1))
                pvT = [psPV.tile([65, 512], fp32, tag="pvT", name="pvT") for _ in range(HPC)]
                for kk in range(4 * m + 4):
                    t = kk - 4 * m  # >= 0 -> this kk-chunk holds the diagonal
                    lo = max(0, t) * 128
                    ksl = slice(128 * kk, 128 * (kk + 1))
                    qsl = slice(512 * m + lo, 512 * (m + 1))
                    for h in range(HPC):
                        tn = "A" if h < 2 else "B"
                        band = slice(64 * (h % 2), 64 * (h % 2) + 64)
                        tp = (64 * (h % 2), 0)
                        sps = psA.tile([128, 512], fp32, tag="ps", name="sps")
                        nc.tensor.matmul(sps[:, lo:512], qk_sb["k" + tn][band, ksl],
                                         qk_sb["q" + tn][band, qsl],
                                         start=True, stop=True, tile_position=tp)
                        at = attn_pool.tile([128, 512], bf16, tag="attn", name="at")
                        nc.scalar.activation(out=at[:, lo:512], in_=sps[:, lo:512],
                                             func=EXP, scale=0.125)
                        if t >= 0:
                            dsl = slice(128 * t, 128 * (t + 1))
                            nc.vector.tensor_tensor(at[:, dsl], at[:, dsl], maskt, op=MULT)
                        nc.tensor.matmul(pvT[h][:, lo:512],
                                         v_sb[kk][:, 65 * h : 65 * h + 65],
                                         at[:, lo:512],
                                         start=(kk == 0), stop=(kk == 4 * m + 3))
                # normalize: row 0 of pvT = denominator (ones-first v layout)
                for h in range(HPC):
                    cix, r0 = h // 2, 64 * (h % 2)
                    rcp = tmp.tile([1, 512], fp32, tag="rcp", name="rcp")
                    nc.vector.reciprocal_approx_fast(rcp, pvT[h][0:1, :])
                    rb = tmp.tile([65, 512], fp32, tag="rb", name="rb")
                    nc.gpsimd.partition_broadcast(rb, rcp[0:1, :])
                    ao = ostage.tile([65, 512], bf16, tag="ao", name="ao")
                    nc.vector.tensor_tensor(ao, pvT[h][0:65, :], rb, op=MULT)
                    nc.sync.dma_start(out=aoT[cix][r0 : r0 + 64, msl], in_=ao[1:65, :])

            def oproj(sc):
                ssl = slice(512 * sc, 512 * (sc + 1))
                for fo in range(F_CH):
                    fsl = slice(128 * fo, 128 * (fo + 1))
                    pw = psA.tile([128, 512], fp32, tag="ps", name="pw")
                    for c in range(2):
                        nc.tensor.matmul(pw, wo[c][:, fsl], aoT[c][:, ssl],
                                         start=(c == 0), stop=(c == 1))
                    ow = ostage.tile([128, 512], fp16, tag="ow", name="ow")
                    if fo % 2 == 0:
                        nc.vector.tensor_copy(ow, pw)
                    else:
                        nc.scalar.copy(ow, pw)
                    nc.sync.dma_start(out=outT_d[fsl, ssl], in_=ow)

            # ---------------- schedule ----------------------------------
            for n in range(N_CH):
                for st in range(4 * n, 4 * n + 4):
                    vproj(st)
                qkproj("k", n)
                qkproj("q", n)
                attn_macro(n)
                if n >= 1:
                    oproj(n - 1)
            oproj(3)

    nc.compile()
    return nc


def _get_nc():
    if "nc" not in _CACHE:
        _CACHE["nc"] = _build_nc()
    return _CACHE["nc"]


def _host_prep(x, positions, Wq, bq, Wk, bk, Wv, bv, Wo, bo):
    """Build the 8 per-core input maps."""
    ts = MAX_WAVELENGTH ** (2.0 * np.arange(HALF, dtype=np.float32) / D)  # [32]
    ii = np.arange(128)
    mask = (ii[:, None] <= ii[None, :]).astype(BF16)
    perm = np.zeros((128, 128), dtype=BF16)
    src = (ii // 64) * 64 + (ii % 64 + 32) % 64
    perm[src, ii] = 1.0

    in_maps = []
    for c in range(NCORES):
        b, g = c // 4, c % 4
        heads = np.arange(4 * g, 4 * g + 4)

        xT = np.empty((F + 1, S), dtype=BF16)
        xT[:F] = x[b].T.astype(BF16)
        xT[F] = 1.0

        wv_e = np.zeros((F + 1, 260), dtype=np.float32)
        for hl, h in enumerate(heads):
            wv_e[F, 65 * hl] = 1.0  # ones column first -> denominator row 0
            wv_e[:F, 65 * hl + 1 : 65 * hl + 65] = Wv[:, 64 * h : 64 * h + 64]
            wv_e[F, 65 * hl + 1 : 65 * hl + 65] = bv[64 * h : 64 * h + 64]

        csl = slice(256 * g, 256 * (g + 1))
        bqk = np.stack([bq[csl][:128], bq[csl][128:], bk[csl][:128], bk[csl][128:]],
                       axis=1).astype(np.float32)  # [128, 4]

        pos = positions[b].astype(np.float32)  # [S]
        ang = pos[None, :] / ts[:, None]  # [32, S]
        cos32, sin32 = np.cos(ang), np.sin(ang)
        cosw = np.tile(np.concatenate([cos32, cos32], 0), (2, 1)).astype(BF16)
        ssgnw = np.tile(np.concatenate([-sin32, sin32], 0), (2, 1)).astype(BF16)

        in_maps.append({
            "xT": xT,
            "wq": Wq[:, csl].astype(BF16),
            "wk": Wk[:, csl].astype(BF16),
            "wv": wv_e.astype(BF16),
            "wo": Wo[64 * heads[0] : 64 * heads[0] + 256, :].astype(BF16),
            "bqk": bqk,
            "cosw": cosw,
            "ssgnw": ssgnw,
            "perm": perm,
            "mask": mask,
        })
    return in_maps


def kernel(x, positions, Wq, bq, Wk, bk, Wv, bv, Wo, bo):
    global LAST_RESULT
    from concourse.bass_utils import run_bass_kernel_spmd

    x = np.asarray(x, dtype=np.float32)
    positions = np.asarray(positions)
    args = [np.asarray(a, dtype=np.float32) for a in (Wq, bq, Wk, bk, Wv, bv, Wo, bo)]
    Wq, bq, Wk, bk, Wv, bv, Wo, bo = args

    nc = _get_nc()
    in_maps = _host_prep(x, positions, Wq, bq, Wk, bk, Wv, bv, Wo, bo)
    try:
        res = run_bass_kernel_spmd(nc, in_maps, core_ids=list(range(NCORES)))
    except ModuleNotFoundError:
        # axon NTFF profiling hook unavailable in this image; run untraced
        os.environ["BASS_NEVER_TRACE"] = "1"
        res = run_bass_kernel_spmd(nc, in_maps, core_ids=list(range(NCORES)))
    LAST_RESULT = res

    out = np.empty((B, S, F), dtype=np.float32)
    for b in range(B):
        acc = np.zeros((F, S), dtype=np.float32)
        for g in range(4):
            acc += res.results[4 * b + g]["outT"].astype(np.float32)
        out[b] = acc.T + bo[None, :]
    return out


# revision 7
# speedup vs baseline: 1.1142x; 1.1047x over previous
"""Multi-head causal attention with RoPE on 8 trn2 cores.

Sharding: core c -> batch b = c // 4, head group g = c % 4 (heads 4g..4g+4).
Each core computes q/k/v projections for its 4 heads, causal attention, and
a partial output-projection (its heads' slice of Wo). The host sums the 4
partials per batch (tensor-parallel unshard) and adds the output bias.

Device layout notes (v2 — PE-utilization-focused rewrite):
  - q/k use an interleaved head layout: tile A = local heads 0,1 and tile B
    = heads 2,3, each head occupying 64 consecutive partitions in natural
    dim order [x1(32); x2(32)].  QK^T is then ONE matmul per head per
    kk-chunk with contraction 64 (vs two K=32 passes before).
  - RoPE: the projection psum is drained to c (bias fused), a permutation
    matmul produces cswap (x1/x2 halves swapped per head), and
    q = c*COS + cswap*SSGN with SSGN = [-sin; +sin] per head: 3 DVE ops.
  - v keeps [s, d] layout; per head the softmax-denominator ones column
    comes FIRST (col 65h), so pv psum row 0 = denominator at partition 0:
    normalization needs no DMA partition hop (reciprocal straight from
    psum row 0, gpsimd broadcast, one multiply).
  - Scores psums are per-head [128, 512] single-bank tiles (4 score bufs +
    4 pv bufs = 8 psum banks) so chunk kk+1's QK can start while chunk
    kk's exps drain: keeps the PE continuously busy (p-state ramp).
  - Phases are interleaved (vproj/kproj/qproj per s-chunk, attention per
    macro, output-projection for macro m emitted after macro m+1) and x is
    DMA'd in s-chunks so the first matmul starts ~1us in.
  - Output is stored as fp16 (halves the output DMA); host sums partials.
"""

import os

import numpy as np
import ml_dtypes

BF16 = ml_dtypes.bfloat16

B, S, F = 2, 2048, 1024
H, D = 16, 64
HALF = D // 2
NCORES = 8
HPC = 4  # heads per core
S_TILES = S // 128  # 16
N_CH = S // 512  # 4  (512-wide column chunks of s)
F_CH = F // 128  # 8
MACROS = 4  # q macro tiles of 512
MAX_WAVELENGTH = 10000.0

_CACHE = {}
LAST_RESULT = None


def _build_nc():
    import concourse.bacc as bacc
    import concourse.tile as tile
    import concourse.mybir as mybir
    import concourse.bass as bass

    fp32 = mybir.dt.float32
    fp16 = mybir.dt.float16
    bf16 = mybir.dt.bfloat16
    MULT = mybir.AluOpType.mult
    EXP = mybir.ActivationFunctionType.Exp
    IDENT = mybir.ActivationFunctionType.Identity

    nc = bacc.Bacc("TRN2", target_bir_lowering=False, debug=False)

    xT_d = nc.dram_tensor("xT", [F + 1, S], bf16, kind="ExternalInput")
    wq_d = nc.dram_tensor("wq", [F, 256], bf16, kind="ExternalInput")
    wk_d = nc.dram_tensor("wk", [F, 256], bf16, kind="ExternalInput")
    wv_d = nc.dram_tensor("wv", [F + 1, 260], bf16, kind="ExternalInput")
    wo_d = nc.dram_tensor("wo", [256, F], bf16, kind="ExternalInput")
    bqk_d = nc.dram_tensor("bqk", [128, 4], fp32, kind="ExternalInput")
    cos_d = nc.dram_tensor("cosw", [128, S], bf16, kind="ExternalInput")
    ssgn_d = nc.dram_tensor("ssgnw", [128, S], bf16, kind="ExternalInput")
    perm_d = nc.dram_tensor("perm", [128, 128], bf16, kind="ExternalInput")
    mask_d = nc.dram_tensor("mask", [128, 128], bf16, kind="ExternalInput")
    outT_d = nc.dram_tensor("outT", [F, S], fp16, kind="ExternalOutput")

    with tile.TileContext(nc) as tc:
        with (
            tc.tile_pool(name="persist", bufs=1) as persist,
            tc.tile_pool(name="tmp", bufs=8) as tmp,
            tc.tile_pool(name="attn", bufs=8) as attn_pool,
            tc.tile_pool(name="ostage", bufs=6) as ostage,
            tc.tile_pool(name="ps", bufs=2, space="PSUM") as psA,
            tc.tile_pool(name="psPV", bufs=4, space="PSUM") as psPV,
        ):
            # ---------------- persistent SBUF tensors -------------------
            xT = [persist.tile([128, S], bf16, tag=f"xT{i}", name=f"xT{i}") for i in range(F_CH)]
            xones = persist.tile([1, S], bf16, tag="xones", name="xones")
            wq = [persist.tile([128, 256], bf16, tag=f"wq{i}", name=f"wq{i}") for i in range(F_CH)]
            wk = [persist.tile([128, 256], bf16, tag=f"wk{i}", name=f"wk{i}") for i in range(F_CH)]
            wv = [persist.tile([128, 260], bf16, tag=f"wv{i}", name=f"wv{i}") for i in range(F_CH)]
            wvb = persist.tile([1, 260], bf16, tag="wvb", name="wvb")
            wo = [persist.tile([128, F], bf16, tag=f"wo{i}", name=f"wo{i}") for i in range(2)]
            bqk = persist.tile([128, 4], fp32, tag="bqk", name="bqk")
            cosw = persist.tile([128, S], bf16, tag="cosw", name="cosw")
            ssgnw = persist.tile([128, S], bf16, tag="ssgnw", name="ssgnw")
            permt = persist.tile([128, 128], bf16, tag="permt", name="permt")
            maskt = persist.tile([128, 128], bf16, tag="maskt", name="maskt")

            # post-RoPE q/k, interleaved layout: tile A heads 0,1 / tile B
            # heads 2,3; head (h%2) at partitions 64*(h%2)..+64, dims natural
            qk_sb = {}
            for nm in ("qA", "qB", "kA", "kB"):
                qk_sb[nm] = persist.tile([128, S], bf16, tag=nm, name=nm)
            # v in [s, d] layout; head h: col 65h = ones, cols 65h+1..+65 = v
            v_sb = [persist.tile([128, 260], bf16, tag=f"v{i}", name=f"v{i}") for i in range(S_TILES)]
            # attention output, [dh, s] layout (head h -> tile h//2 rows 64*(h%2))
            aoT = [persist.tile([128, S], bf16, tag=f"aoT{i}", name=f"aoT{i}") for i in range(2)]

            # ---------------- input DMA, ordered by first use -----------
            for i in range(F_CH):
                nc.sync.dma_start(out=wv[i], in_=wv_d[128 * i : 128 * (i + 1), :])
            nc.sync.dma_start(out=wvb, in_=wv_d[F : F + 1, :])
            nc.sync.dma_start(out=xones, in_=xT_d[F : F + 1, :])
            nc.sync.dma_start(out=bqk, in_=bqk_d[:, :])
            nc.sync.dma_start(out=permt, in_=perm_d[:, :])
            nc.sync.dma_start(out=maskt, in_=mask_d[:, :])
            # x s-chunk 0, then q/k weights + rope tables, then the rest
            for i in range(F_CH):
                nc.sync.dma_start(out=xT[i][:, 0:512], in_=xT_d[128 * i : 128 * (i + 1), 0:512])
            for i in range(F_CH):
                nc.sync.dma_start(out=wk[i], in_=wk_d[128 * i : 128 * (i + 1), :])
                nc.sync.dma_start(out=wq[i], in_=wq_d[128 * i : 128 * (i + 1), :])
            nc.sync.dma_start(out=cosw, in_=cos_d[:, :])
            nc.sync.dma_start(out=ssgnw, in_=ssgn_d[:, :])
            for sc in range(1, N_CH):
                ssl = slice(512 * sc, 512 * (sc + 1))
                for i in range(F_CH):
                    nc.sync.dma_start(out=xT[i][:, ssl], in_=xT_d[128 * i : 128 * (i + 1), ssl])
            for i in range(2):
                nc.sync.dma_start(out=wo[i], in_=wo_d[128 * i : 128 * (i + 1), :])

            # ---------------- phase emitters ----------------------------
            def vproj(st):
                ps = psA.tile([128, 260], fp32, tag="ps", name="psv")
                sl = slice(128 * st, 128 * (st + 1))
                for kc in range(F_CH):
                    nc.tensor.matmul(ps, xT[kc][:, sl], wv[kc], start=(kc == 0), stop=False)
                nc.tensor.matmul(ps, xones[:, sl], wvb, start=False, stop=True)
                nc.scalar.copy(v_sb[st], ps)

            def qkproj(which, n):
                w_sb = wq if which == "q" else wk
                bcol = 0 if which == "q" else 2
                nsl = slice(512 * n, 512 * (n + 1))
                for ti, tn in enumerate("AB"):
                    out = qk_sb[which + tn]
                    csl = slice(128 * ti, 128 * (ti + 1))
                    ps = psA.tile([128, 512], fp32, tag="ps", name="psp")
                    for kc in range(F_CH):
                        nc.tensor.matmul(ps, w_sb[kc][:, csl], xT[kc][:, nsl],
                                         start=(kc == 0), stop=(kc == F_CH - 1))
                    c = tmp.tile([128, 512], bf16, tag="rope", name="c")
                    nc.scalar.activation(c, ps, func=IDENT,
                                         bias=bqk[:, bcol + ti : bcol + ti + 1])
                    pss = psA.tile([128, 512], fp32, tag="ps", name="pss")
                    nc.tensor.matmul(pss, permt, c, start=True, stop=True)
                    t1 = tmp.tile([128, 512], bf16, tag="rope", name="t1")
                    t2 = tmp.tile([128, 512], bf16, tag="rope", name="t2")
                    nc.vector.tensor_mul(t1, c, cosw[:, nsl])
                    nc.vector.tensor_tensor(t2, pss, ssgnw[:, nsl], op=MULT)
                    nc.vector.tensor_add(out[:, nsl], t1, t2)

            def attn_macro(m):
                msl = slice(512 * m, 512 * (m + 1))
                pvT = [psPV.tile([65, 512], fp32, tag="pvT", name="pvT") for _ in range(HPC)]
                # superchunks: two 128-wide kk chunks share one [128, 1024]
                # score psum per head -> one QK sync group, one exp op, and
                # the two PV passes share a single wait.
                for sc in range(2 * m + 2):
                    kks = (2 * sc, 2 * sc + 1)
                    ts = [kk - 4 * m for kk in kks]
                    los = [max(0, t) * 128 for t in ts]
                    diag = ts[1] >= 0
                    for h in range(HPC):
                        tn = "A" if h < 2 else "B"
                        band = slice(64 * (h % 2), 64 * (h % 2) + 64)
                        tp = (64 * (h % 2), 0)
                        sps = psA.tile([128, 1024], fp32, tag="ps", name="sps")
                        for i, kk in enumerate(kks):
                            lo = los[i]
                            ksl = slice(128 * kk, 128 * (kk + 1))
                            qsl = slice(512 * m + lo, 512 * (m + 1))
                            nc.tensor.matmul(sps[:, 512 * i + lo : 512 * (i + 1)],
                                             qk_sb["k" + tn][band, ksl],
                                             qk_sb["q" + tn][band, qsl],
                                             start=True, stop=(i == 1),
                                             tile_position=tp, skip_group_check=True)
                        at = attn_pool.tile([128, 1024], bf16, tag="attn", name="at")
                        if not diag:
                            nc.scalar.activation(out=at, in_=sps, func=EXP, scale=0.125)
                        else:
                            for i in range(2):
                                lo = los[i]
                                hsl = slice(512 * i + lo, 512 * (i + 1))
                                nc.scalar.activation(out=at[:, hsl], in_=sps[:, hsl],
                                                     func=EXP, scale=0.125)
                        for i in range(2):
                            if ts[i] >= 0:
                                dsl = slice(512 * i + 128 * ts[i], 512 * i + 128 * (ts[i] + 1))
                                nc.vector.tensor_tensor(at[:, dsl], at[:, dsl], maskt, op=MULT)
                        for i, kk in enumerate(kks):
                            lo = los[i]
                            nc.tensor.matmul(pvT[h][:, lo:512],
                                             v_sb[kk][:, 65 * h : 65 * h + 65],
                                             at[:, 512 * i + lo : 512 * (i + 1)],
                                             start=(kk == 0), stop=(kk == 4 * m + 3))
                # normalize: row 0 of pvT = denominator (ones-first v layout)
                for h in range(HPC):
                    cix, r0 = h // 2, 64 * (h % 2)
                    rcp = tmp.tile([1, 512], fp32, tag="rcp", name="rcp")
                    nc.vector.reciprocal_approx_fast(rcp, pvT[h][0:1, :])
                    rb = tmp.tile([65, 512], fp32, tag="rb", name="rb")
                    nc.gpsimd.partition_broadcast(rb, rcp[0:1, :])
                    ao = ostage.tile([65, 512], bf16, tag="ao", name="ao")
                    nc.vector.tensor_tensor(ao, pvT[h][0:65, :], rb, op=MULT)
                    nc.sync.dma_start(out=aoT[cix][r0 : r0 + 64, msl], in_=ao[1:65, :])

            def oproj(sc):
                ssl = slice(512 * sc, 512 * (sc + 1))
                for fo in range(F_CH):
                    fsl = slice(128 * fo, 128 * (fo + 1))
                    pw = psA.tile([128, 512], fp32, tag="ps", name="pw")
                    for c in range(2):
                        nc.tensor.matmul(pw, wo[c][:, fsl], aoT[c][:, ssl],
                                         start=(c == 0), stop=(c == 1))
                    ow = ostage.tile([128, 512], fp16, tag="ow", name="ow")
                    if fo % 2 == 0:
                        nc.vector.tensor_copy(ow, pw)
                    else:
                        nc.scalar.copy(ow, pw)
                    nc.sync.dma_start(out=outT_d[fsl, ssl], in_=ow)

            # ---------------- schedule ----------------------------------
            for n in range(N_CH):
                for st in range(4 * n, 4 * n + 4):
                    vproj(st)
                qkproj("k", n)
                qkproj("q", n)
                attn_macro(n)
                if n >= 1:
                    oproj(n - 1)
            oproj(3)

    nc.compile()
    return nc


def _get_nc():
    if "nc" not in _CACHE:
        _CACHE["nc"] = _build_nc()
    return _CACHE["nc"]


def _host_prep(x, positions, Wq, bq, Wk, bk, Wv, bv, Wo, bo):
    """Build the 8 per-core input maps."""
    ts = MAX_WAVELENGTH ** (2.0 * np.arange(HALF, dtype=np.float32) / D)  # [32]
    ii = np.arange(128)
    mask = (ii[:, None] <= ii[None, :]).astype(BF16)
    perm = np.zeros((128, 128), dtype=BF16)
    src = (ii // 64) * 64 + (ii % 64 + 32) % 64
    perm[src, ii] = 1.0

    in_maps = []
    for c in range(NCORES):
        b, g = c // 4, c % 4
        heads = np.arange(4 * g, 4 * g + 4)

        xT = np.empty((F + 1, S), dtype=BF16)
        xT[:F] = x[b].T.astype(BF16)
        xT[F] = 1.0

        wv_e = np.zeros((F + 1, 260), dtype=np.float32)
        for hl, h in enumerate(heads):
            wv_e[F, 65 * hl] = 1.0  # ones column first -> denominator row 0
            wv_e[:F, 65 * hl + 1 : 65 * hl + 65] = Wv[:, 64 * h : 64 * h + 64]
            wv_e[F, 65 * hl + 1 : 65 * hl + 65] = bv[64 * h : 64 * h + 64]

        csl = slice(256 * g, 256 * (g + 1))
        bqk = np.stack([bq[csl][:128], bq[csl][128:], bk[csl][:128], bk[csl][128:]],
                       axis=1).astype(np.float32)  # [128, 4]

        pos = positions[b].astype(np.float32)  # [S]
        ang = pos[None, :] / ts[:, None]  # [32, S]
        cos32, sin32 = np.cos(ang), np.sin(ang)
        cosw = np.tile(np.concatenate([cos32, cos32], 0), (2, 1)).astype(BF16)
        ssgnw = np.tile(np.concatenate([-sin32, sin32], 0), (2, 1)).astype(BF16)

        in_maps.append({
            "xT": xT,
            "wq": Wq[:, csl].astype(BF16),
            "wk": Wk[:, csl].astype(BF16),
            "wv": wv_e.astype(BF16),
            "wo": Wo[64 * heads[0] : 64 * heads[0] + 256, :].astype(BF16),
            "bqk": bqk,
            "cosw": cosw,
            "ssgnw": ssgnw,
            "perm": perm,
            "mask": mask,
        })
    return in_maps


def kernel(x, positions, Wq, bq, Wk, bk, Wv, bv, Wo, bo):
    global LAST_RESULT
    from concourse.bass_utils import run_bass_kernel_spmd

    x = np.asarray(x, dtype=np.float32)
    positions = np.asarray(positions)
    args = [np.asarray(a, dtype=np.float32) for a in (Wq, bq, Wk, bk, Wv, bv, Wo, bo)]
    Wq, bq, Wk, bk, Wv, bv, Wo, bo = args

    nc = _get_nc()
    in_maps = _host_prep(x, positions, Wq, bq, Wk, bk, Wv, bv, Wo, bo)
    try:
        res = run_bass_kernel_spmd(nc, in_maps, core_ids=list(range(NCORES)))
    except ModuleNotFoundError:
        # axon NTFF profiling hook unavailable in this image; run untraced
        os.environ["BASS_NEVER_TRACE"] = "1"
        res = run_bass_kernel_spmd(nc, in_maps, core_ids=list(range(NCORES)))
    LAST_RESULT = res

    out = np.empty((B, S, F), dtype=np.float32)
    for b in range(B):
        acc = np.zeros((F, S), dtype=np.float32)
        for g in range(4):
            acc += res.results[4 * b + g]["outT"].astype(np.float32)
        out[b] = acc.T + bo[None, :]
    return out


# revision 11
# speedup vs baseline: 1.3743x; 1.2334x over previous
"""Multi-head causal attention with RoPE on 8 trn2 cores.

Sharding: core c -> batch b = c // 4, head group g = c % 4 (heads 4g..4g+4).
Each core computes q/k/v projections for its 4 heads, causal attention, and
a partial output-projection (its heads' slice of Wo). The host sums the 4
partials per batch (tensor-parallel unshard) and adds the output bias.

Device layout notes (v2 — PE-utilization-focused rewrite):
  - q/k use an interleaved head layout: tile A = local heads 0,1 and tile B
    = heads 2,3, each head occupying 64 consecutive partitions in natural
    dim order [x1(32); x2(32)].  QK^T is then ONE matmul per head per
    kk-chunk with contraction 64 (vs two K=32 passes before).
  - RoPE: the projection psum is drained to c (bias fused), a permutation
    matmul produces cswap (x1/x2 halves swapped per head), and
    q = c*COS + cswap*SSGN with SSGN = [-sin; +sin] per head: 3 DVE ops.
  - v keeps [s, d] layout; per head the softmax-denominator ones column
    comes FIRST (col 65h), so pv psum row 0 = denominator at partition 0:
    normalization needs no DMA partition hop (reciprocal straight from
    psum row 0, gpsimd broadcast, one multiply).
  - Scores psums are per-head [128, 512] single-bank tiles (4 score bufs +
    4 pv bufs = 8 psum banks) so chunk kk+1's QK can start while chunk
    kk's exps drain: keeps the PE continuously busy (p-state ramp).
  - Phases are interleaved (vproj/kproj/qproj per s-chunk, attention per
    macro, output-projection for macro m emitted after macro m+1) and x is
    DMA'd in s-chunks so the first matmul starts ~1us in.
  - Output is stored as fp16 (halves the output DMA); host sums partials.
"""

import os

import numpy as np
import ml_dtypes

BF16 = ml_dtypes.bfloat16

B, S, F = 2, 2048, 1024
H, D = 16, 64
HALF = D // 2
NCORES = 8
HPC = 4  # heads per core
S_TILES = S // 128  # 16
N_CH = S // 512  # 4  (512-wide column chunks of s)
F_CH = F // 128  # 8
MACROS = 4  # q macro tiles of 512
MAX_WAVELENGTH = 10000.0

_CACHE = {}
LAST_RESULT = None


def _build_nc():
    import concourse.bacc as bacc
    import concourse.tile as tile
    import concourse.mybir as mybir
    import concourse.bass as bass

    fp32 = mybir.dt.float32
    fp16 = mybir.dt.float16
    bf16 = mybir.dt.bfloat16
    MULT = mybir.AluOpType.mult
    EXP = mybir.ActivationFunctionType.Exp
    IDENT = mybir.ActivationFunctionType.Identity

    nc = bacc.Bacc("TRN2", target_bir_lowering=False, debug=False)

    xT_d = nc.dram_tensor("xT", [F + 1, S], bf16, kind="ExternalInput")
    wq_d = nc.dram_tensor("wq", [F, 256], bf16, kind="ExternalInput")
    wk_d = nc.dram_tensor("wk", [F, 256], bf16, kind="ExternalInput")
    wv_d = nc.dram_tensor("wv", [F + 1, 260], bf16, kind="ExternalInput")
    wo_d = nc.dram_tensor("wo", [256, F], bf16, kind="ExternalInput")
    bqk_d = nc.dram_tensor("bqk", [128, 4], fp32, kind="ExternalInput")
    cos_d = nc.dram_tensor("cosw", [128, S], bf16, kind="ExternalInput")
    ssgn_d = nc.dram_tensor("ssgnw", [128, S], bf16, kind="ExternalInput")
    perm_d = nc.dram_tensor("perm", [128, 128], bf16, kind="ExternalInput")
    mask_d = nc.dram_tensor("mask", [128, 128], bf16, kind="ExternalInput")
    outT_d = nc.dram_tensor("outT", [F, S], fp16, kind="ExternalOutput")

    with tile.TileContext(nc) as tc:
        with (
            tc.tile_pool(name="persist", bufs=1) as persist,
            tc.tile_pool(name="tmp", bufs=8) as tmp,
            tc.tile_pool(name="attn", bufs=8) as attn_pool,
            tc.tile_pool(name="ostage", bufs=6) as ostage,
            tc.tile_pool(name="ps", bufs=4, space="PSUM") as psA,
            tc.tile_pool(name="psPV", bufs=4, space="PSUM") as psPV,
        ):
            # ---------------- persistent SBUF tensors -------------------
            xT = [persist.tile([128, S], bf16, tag=f"xT{i}", name=f"xT{i}") for i in range(F_CH)]
            xones = persist.tile([1, S], bf16, tag="xones", name="xones")
            wq = [persist.tile([128, 256], bf16, tag=f"wq{i}", name=f"wq{i}") for i in range(F_CH)]
            wk = [persist.tile([128, 256], bf16, tag=f"wk{i}", name=f"wk{i}") for i in range(F_CH)]
            wv = [persist.tile([128, 260], bf16, tag=f"wv{i}", name=f"wv{i}") for i in range(F_CH)]
            wvb = persist.tile([1, 260], bf16, tag="wvb", name="wvb")
            wo = [persist.tile([128, F], bf16, tag=f"wo{i}", name=f"wo{i}") for i in range(2)]
            bqk = persist.tile([128, 4], fp32, tag="bqk", name="bqk")
            cosw = persist.tile([128, S], bf16, tag="cosw", name="cosw")
            ssgnw = persist.tile([128, S], bf16, tag="ssgnw", name="ssgnw")
            permt = persist.tile([128, 128], bf16, tag="permt", name="permt")
            maskt = persist.tile([128, 128], bf16, tag="maskt", name="maskt")

            # post-RoPE q/k, interleaved layout: tile A heads 0,1 / tile B
            # heads 2,3; head (h%2) at partitions 64*(h%2)..+64, dims natural
            qk_sb = {}
            for nm in ("qA", "qB", "kA", "kB"):
                qk_sb[nm] = persist.tile([128, S], bf16, tag=nm, name=nm)
            # v in [s, d] layout; head h: col 65h = ones, cols 65h+1..+65 = v
            v_sb = [persist.tile([128, 260], bf16, tag=f"v{i}", name=f"v{i}") for i in range(S_TILES)]
            # attention output, [dh, s] layout (head h -> tile h//2 rows 64*(h%2))
            aoT = [persist.tile([128, S], bf16, tag=f"aoT{i}", name=f"aoT{i}") for i in range(2)]

            # ---------------- input DMA, ordered by first use -----------
            for i in range(F_CH):
                nc.sync.dma_start(out=wv[i], in_=wv_d[128 * i : 128 * (i + 1), :])
            nc.sync.dma_start(out=wvb, in_=wv_d[F : F + 1, :])
            nc.sync.dma_start(out=xones, in_=xT_d[F : F + 1, :])
            nc.sync.dma_start(out=bqk, in_=bqk_d[:, :])
            nc.sync.dma_start(out=permt, in_=perm_d[:, :])
            nc.sync.dma_start(out=maskt, in_=mask_d[:, :])
            # x s-chunk 0, then q/k weights + rope tables, then the rest
            for i in range(F_CH):
                nc.sync.dma_start(out=xT[i][:, 0:512], in_=xT_d[128 * i : 128 * (i + 1), 0:512])
            for i in range(F_CH):
                nc.sync.dma_start(out=wk[i], in_=wk_d[128 * i : 128 * (i + 1), :])
                nc.sync.dma_start(out=wq[i], in_=wq_d[128 * i : 128 * (i + 1), :])
            nc.sync.dma_start(out=cosw, in_=cos_d[:, :])
            nc.sync.dma_start(out=ssgnw, in_=ssgn_d[:, :])
            for sc in range(1, N_CH):
                ssl = slice(512 * sc, 512 * (sc + 1))
                for i in range(F_CH):
                    nc.sync.dma_start(out=xT[i][:, ssl], in_=xT_d[128 * i : 128 * (i + 1), ssl])
            for i in range(2):
                nc.sync.dma_start(out=wo[i], in_=wo_d[128 * i : 128 * (i + 1), :])

            # ---------------- phase emitters ----------------------------
            def vproj(st):
                ps = psA.tile([128, 260], fp32, tag="ps", name="psv")
                sl = slice(128 * st, 128 * (st + 1))
                for kc in range(F_CH):
                    nc.tensor.matmul(ps, xT[kc][:, sl], wv[kc], start=(kc == 0), stop=False)
                nc.tensor.matmul(ps, xones[:, sl], wvb, start=False, stop=True)
                nc.scalar.copy(v_sb[st], ps)

            def qkproj(which, n, vp=None):
                w_sb = wq if which == "q" else wk
                bcol = 0 if which == "q" else 2
                nsl = slice(512 * n, 512 * (n + 1))
                for ti, tn in enumerate("AB"):
                    if vp is not None and ti == 1:
                        vproj(vp)
                    out = qk_sb[which + tn]
                    csl = slice(128 * ti, 128 * (ti + 1))
                    ps = psA.tile([128, 512], fp32, tag="ps", name="psp")
                    for kc in range(F_CH):
                        nc.tensor.matmul(ps, w_sb[kc][:, csl], xT[kc][:, nsl],
                                         start=(kc == 0), stop=(kc == F_CH - 1))
                    c = tmp.tile([128, 512], bf16, tag="rope", name="c")
                    nc.scalar.activation(c, ps, func=IDENT,
                                         bias=bqk[:, bcol + ti : bcol + ti + 1])
                    pss = psA.tile([128, 512], fp32, tag="ps", name="pss")
                    nc.tensor.matmul(pss, permt, c, start=True, stop=True)
                    t1 = tmp.tile([128, 512], bf16, tag="rope", name="t1")
                    t2 = tmp.tile([128, 512], bf16, tag="rope", name="t2")
                    nc.vector.tensor_mul(t1, c, cosw[:, nsl])
                    nc.vector.tensor_tensor(t2, pss, ssgnw[:, nsl], op=MULT)
                    nc.vector.tensor_add(out[:, nsl], t1, t2)

            def attn_macro(m, filler=None):
                msl = slice(512 * m, 512 * (m + 1))
                pvT = [psPV.tile([65, 512], fp32, tag="pvT", name="pvT") for _ in range(HPC)]
                for kk in range(4 * m + 4):
                    t = kk - 4 * m  # >= 0 -> this kk-chunk holds the diagonal
                    lo = max(0, t) * 128
                    ksl = slice(128 * kk, 128 * (kk + 1))
                    qsl = slice(512 * m + lo, 512 * (m + 1))
                    # all 4 QK matmuls first: heads alternate row strips
                    # (0,0)/(64,0) so adjacent matmuls run 2-way concurrent
                    # in the 64-row-tiled PE array.
                    sps = []
                    for h in range(HPC):
                        tn = "A" if h < 2 else "B"
                        band = slice(64 * (h % 2), 64 * (h % 2) + 64)
                        tp = (64 * (h % 2), 0)
                        ps = psA.tile([128, 512], fp32, tag="ps", name="sps")
                        nc.tensor.matmul(ps[:, lo:512], qk_sb["k" + tn][band, ksl],
                                         qk_sb["q" + tn][band, qsl],
                                         start=True, stop=True, tile_position=tp)
                        sps.append(ps)
                    ats = []
                    for h in range(HPC):
                        at = attn_pool.tile([128, 512], bf16, tag="attn", name="at")
                        nc.scalar.activation(out=at[:, lo:512], in_=sps[h][:, lo:512],
                                             func=EXP, scale=0.125)
                        if t >= 0:
                            dsl = slice(128 * t, 128 * (t + 1))
                            nc.vector.tensor_tensor(at[:, dsl], at[:, dsl], maskt, op=MULT)
                        ats.append(at)
                    for h in range(HPC):
                        nc.tensor.matmul(pvT[h][:, lo:512],
                                         v_sb[kk][:, 65 * h : 65 * h + 65],
                                         ats[h][:, lo:512],
                                         start=(kk == 0), stop=(kk == 4 * m + 3))
                    # PE filler between chunks (ACT-bound phase): o-proj
                    # groups for the previous macro's s-chunk.
                    if filler is not None and kk < len(filler):
                        filler[kk]()
                # normalize: row 0 of pvT = denominator (ones-first v layout)
                for h in range(HPC):
                    cix, r0 = h // 2, 64 * (h % 2)
                    rcp = tmp.tile([1, 512], fp32, tag="rcp", name="rcp")
                    nc.vector.reciprocal_approx_fast(rcp, pvT[h][0:1, :])
                    rb = tmp.tile([65, 512], fp32, tag="rb", name="rb")
                    nc.gpsimd.partition_broadcast(rb, rcp[0:1, :])
                    ao = ostage.tile([65, 512], bf16, tag="ao", name="ao")
                    nc.vector.tensor_tensor(ao, pvT[h][0:65, :], rb, op=MULT)
                    nc.sync.dma_start(out=aoT[cix][r0 : r0 + 64, msl], in_=ao[1:65, :])

            def oproj_fo(sc, fo):
                ssl = slice(512 * sc, 512 * (sc + 1))
                fsl = slice(128 * fo, 128 * (fo + 1))
                pw = psA.tile([128, 512], fp32, tag="ps", name="pw")
                for c in range(2):
                    nc.tensor.matmul(pw, wo[c][:, fsl], aoT[c][:, ssl],
                                     start=(c == 0), stop=(c == 1))
                ow = ostage.tile([128, 512], fp16, tag="ow", name="ow")
                nc.vector.tensor_copy(ow, pw)
                nc.sync.dma_start(out=outT_d[fsl, ssl], in_=ow)

            # ---------------- schedule ----------------------------------
            # vproj interleaved between qkproj groups to fill PE while the
            # psum ring waits on ACT drains; o-proj of macro m-1 spread
            # through attention macro m as PE filler.
            for n in range(N_CH):
                sts = list(range(4 * n, 4 * n + 4))
                vproj(sts[0])
                qkproj("k", n, vp=sts[1])
                vproj(sts[2])
                qkproj("q", n, vp=sts[3])
                fill = None
                if n >= 1:
                    fill = [(lambda sc=n - 1, fo=f: oproj_fo(sc, fo)) for f in range(F_CH)]
                attn_macro(n, filler=fill)
            for fo in range(F_CH):
                oproj_fo(3, fo)

    nc.compile()
    return nc


def _get_nc():
    if "nc" not in _CACHE:
        _CACHE["nc"] = _build_nc()
    return _CACHE["nc"]


def _host_prep(x, positions, Wq, bq, Wk, bk, Wv, bv, Wo, bo):
    """Build the 8 per-core input maps."""
    ts = MAX_WAVELENGTH ** (2.0 * np.arange(HALF, dtype=np.float32) / D)  # [32]
    ii = np.arange(128)
    mask = (ii[:, None] <= ii[None, :]).astype(BF16)
    perm = np.zeros((128, 128), dtype=BF16)
    src = (ii // 64) * 64 + (ii % 64 + 32) % 64
    perm[src, ii] = 1.0

    in_maps = []
    for c in range(NCORES):
        b, g = c // 4, c % 4
        heads = np.arange(4 * g, 4 * g + 4)

        xT = np.empty((F + 1, S), dtype=BF16)
        xT[:F] = x[b].T.astype(BF16)
        xT[F] = 1.0

        wv_e = np.zeros((F + 1, 260), dtype=np.float32)
        for hl, h in enumerate(heads):
            wv_e[F, 65 * hl] = 1.0  # ones column first -> denominator row 0
            wv_e[:F, 65 * hl + 1 : 65 * hl + 65] = Wv[:, 64 * h : 64 * h + 64]
            wv_e[F, 65 * hl + 1 : 65 * hl + 65] = bv[64 * h : 64 * h + 64]

        csl = slice(256 * g, 256 * (g + 1))
        bqk = np.stack([bq[csl][:128], bq[csl][128:], bk[csl][:128], bk[csl][128:]],
                       axis=1).astype(np.float32)  # [128, 4]

        pos = positions[b].astype(np.float32)  # [S]
        ang = pos[None, :] / ts[:, None]  # [32, S]
        cos32, sin32 = np.cos(ang), np.sin(ang)
        cosw = np.tile(np.concatenate([cos32, cos32], 0), (2, 1)).astype(BF16)
        ssgnw = np.tile(np.concatenate([-sin32, sin32], 0), (2, 1)).astype(BF16)

        in_maps.append({
            "xT": xT,
            "wq": Wq[:, csl].astype(BF16),
            "wk": Wk[:, csl].astype(BF16),
            "wv": wv_e.astype(BF16),
            "wo": Wo[64 * heads[0] : 64 * heads[0] + 256, :].astype(BF16),
            "bqk": bqk,
            "cosw": cosw,
            "ssgnw": ssgnw,
            "perm": perm,
            "mask": mask,
        })
    return in_maps


def kernel(x, positions, Wq, bq, Wk, bk, Wv, bv, Wo, bo):
    global LAST_RESULT
    from concourse.bass_utils import run_bass_kernel_spmd

    x = np.asarray(x, dtype=np.float32)
    positions = np.asarray(positions)
    args = [np.asarray(a, dtype=np.float32) for a in (Wq, bq, Wk, bk, Wv, bv, Wo, bo)]
    Wq, bq, Wk, bk, Wv, bv, Wo, bo = args

    nc = _get_nc()
    in_maps = _host_prep(x, positions, Wq, bq, Wk, bk, Wv, bv, Wo, bo)
    try:
        res = run_bass_kernel_spmd(nc, in_maps, core_ids=list(range(NCORES)))
    except ModuleNotFoundError:
        # axon NTFF profiling hook unavailable in this image; run untraced
        os.environ["BASS_NEVER_TRACE"] = "1"
        res = run_bass_kernel_spmd(nc, in_maps, core_ids=list(range(NCORES)))
    LAST_RESULT = res

    out = np.empty((B, S, F), dtype=np.float32)
    for b in range(B):
        acc = np.zeros((F, S), dtype=np.float32)
        for g in range(4):
            acc += res.results[4 * b + g]["outT"].astype(np.float32)
        out[b] = acc.T + bo[None, :]
    return out
